# revision 14
# baseline (speedup 1.0000x reference)
"""ConvLSTM cell kernel for Trainium2 (8 NeuronCores).

Sharding: data-parallel over batch B=4 x spatial split of H=64 into 2 halves
(8 shards). The recurrence prevents sharding T. Each core computes its half
with a shrinking row margin (47-t rows at step t) so no cross-core
communication is needed. Bottom halves are row-flipped on the host (x rows
flipped + conv kernel dy-flipped) so a single SPMD program serves all 8 cores.

Compute scheme (per core, per step):
  - h state lives in SBUF twice: bf16 `hw` [128, 50, 64] (parts 0:64 row r =
    Hs[r-1], parts 64:128 row r = Hs[r], Hs = 2*h) for output + conversion,
    and fp8e4 `hb` [128, 50, 80] (same row mapping, zero pad col 0, zero
    cols 65..79) used as conv rhs. Conv weights are pre-scaled by 8
    (= (w/2)*16) in fp8, x pre-scaled by 16 in bf16 and pre-padded to the
    80-col layout on the host, so psum = 16*z exactly.
  - 3x3 conv via fp8 DoubleRow matmuls: contraction 128 partitions x Ko=2
    (Ko stride = one 80-col padded row). Slots: (p<64,j0)=dy0, (p<64,j1)=dy1,
    (p>=64,j0)=dup(zero weight), (p>=64,j1)=dy2. One matmul per dx (3) with a
    flat 480-wide rhs span (6 output rows x 80 incl. junk pad cols), plus one
    bf16 identity matmul adding the padded x.
  - Gates: tau0 psum = [i;f] -> sigmoid (scale 1/16). tau1 psum = [g;o] ->
    tanh with per-partition scale [1/16; 1/32] giving [g; so=tanh(zo/2)].
    ACT reads psum via 4D strided APs (two 6-row subs per 2-bank psum tile),
    writing compact bf16 gate tiles. DVE: P = [i;f]*[g;c] (tensor_tensor, 2x
    mode), copy+add folds to c_new = i*g + f*c, ACT tanh(c_new), then one
    scalar_tensor_tensor Hs = (so+1)*tanh(c) = 2*o*tanh(c). Host multiplies
    the output by 0.5.
  - GpSimd converts [h;h_shifted] bf16 -> fp8 conv tiles; a SBUF->SBUF DMA
    builds the row-shifted bf16 half.
"""

import sys
import dataclasses

sys.path.insert(0, "/opt/trn_rl_repo")

import numpy as np
from ml_dtypes import bfloat16, float8_e4m3

HIDDEN = 64
T_STEPS = 16
B = 4
H = 64
W = 64
ROWS = 48         # per-core x rows (32 owned + 16 margin)
OWN = 32
WP8 = 80          # padded row width (DoubleRow Ko stride must be %16 bytes)
HR8 = 50          # h tile rows (top pad + 48 + bottom guard)
SUB = 6           # output rows per matmul (6*80=480 <= 512 fp32 psum bank)
CHUNK = 4         # subs per elementwise chunk (24 rows)

_CACHE = {}


def _replace_ap(ap, new_dims, extra_offset=0):
    a = ap.copy()
    return dataclasses.replace(a, ap=type(a.ap)(new_dims),
                               offset=a.offset + extra_offset)


def _build_nc():
    from concourse import bacc, mybir
    from concourse.tile import TileContext

    dt = mybir.dt
    Alu = mybir.AluOpType
    Act = mybir.ActivationFunctionType
    PM = mybir.MatmulPerfMode

    nc = bacc.Bacc(None, target_bir_lowering=False)

    x_in = nc.dram_tensor("x", [T_STEPS, 2, 128, ROWS * WP8], dt.bfloat16,
                          kind="ExternalInput")
    w_in = nc.dram_tensor("wdr", [128, 6 * 2 * 128], dt.float8e4,
                          kind="ExternalInput")
    id_in = nc.dram_tensor("ident", [128, 128], dt.bfloat16,
                           kind="ExternalInput")
    sc_in = nc.dram_tensor("scB", [128, 1], dt.float32,
                           kind="ExternalInput")
    hout = nc.dram_tensor("hout", [T_STEPS, 64, OWN * W], dt.bfloat16,
                          kind="ExternalOutput")

    NSUBMAX = ROWS // SUB            # 8
    GW = NSUBMAX * SUB * W           # 3072 gate-tile width

    with TileContext(nc) as tc:
        with (
            tc.tile_pool(name="const", bufs=1) as cpool,
            tc.tile_pool(name="state", bufs=1) as spool,
            tc.tile_pool(name="xload", bufs=2) as xpool,
            tc.tile_pool(name="work", bufs=2) as wpool,
            tc.tile_pool(name="ps", bufs=2, space="PSUM") as psp,
        ):
            w_sb = cpool.tile([128, 6, 2, 128], dt.float8e4, tag="w")
            id_sb = cpool.tile([128, 128], dt.bfloat16, tag="id")
            sc_sb = cpool.tile([128, 1], dt.float32, tag="sc")
            nc.sync.dma_start(out=w_sb[:], in_=w_in[:].rearrange(
                "p (a b c) -> p a b c", a=6, b=2))
            nc.sync.dma_start(out=id_sb[:], in_=id_in[:])
            nc.sync.dma_start(out=sc_sb[:], in_=sc_in[:])

            # fp8 conv-input state, ping-pong across steps
            hb = [spool.tile([128, HR8, WP8], dt.float8e4, tag=f"hb{k}",
                             name=f"hb{k}") for k in range(2)]
            nc.gpsimd.memset(hb[0][:], 0.0)
            nc.gpsimd.memset(hb[1][:], 0.0)
            # bf16 h tiles (ping-pong): parts 0:64 row r = Hs[r-1],
            # parts 64:128 row r = Hs[r]
            hwp = [spool.tile([128, HR8, W], dt.bfloat16, tag=f"hw{k}",
                              name=f"hw{k}") for k in range(2)]
            nc.gpsimd.memset(hwp[0][:], 0.0)
            nc.gpsimd.memset(hwp[1][:], 0.0)
            # c state (parts 64:128) + per-step g scratch (parts 0:64)
            cs = spool.tile([128, GW], dt.bfloat16, tag="cs")
            nc.gpsimd.memset(cs[:], 0.0)

            for t in range(T_STEPS):
                R = 47 - t
                nsub = -(-R // SUB)
                nrows = nsub * SUB
                hbr = hb[(t + 1) % 2]   # conv input (written at t-1)
                hbw_t = hb[t % 2]       # conv input for t+1 (written now)
                hw = hwp[t % 2]

                xt = []
                for half in range(2):
                    xti = xpool.tile([128, ROWS * WP8], dt.bfloat16,
                                     tag=f"x{half}", name=f"x{half}")
                    nc.sync.dma_start(out=xti[:, : nrows * WP8],
                                      in_=x_in[t, half][:, : nrows * WP8])
                    xt.append(xti)

                sa = wpool.tile([128, GW], dt.bfloat16, tag="sa", name="sa")
                sb = wpool.tile([128, GW], dt.bfloat16, tag="sb", name="sb")
                pp = wpool.tile([128, GW], dt.bfloat16, tag="pp", name="pp")
                qq = wpool.tile([128, GW], dt.bfloat16, tag="qq", name="qq")
                tch = wpool.tile([128, GW], dt.bfloat16, tag="tc", name="tc")

                # spans: first two small (12 rows) to release the next
                # step's rows early, remainder bigger for op efficiency
                if nsub > 4:
                    spans = [(0, 2), (2, 4), (4, nsub)]
                else:
                    spans = [(0, 2), (2, nsub)]
                for sp0, sp1 in spans:
                    pairs = [(pa, min(pa + 2, sp1))
                             for pa in range(sp0, sp1, 2)]
                    ptiles = {}
                    for tau in range(2):
                        for pa, pb in pairs:
                            ptiles[tau, pa] = psp.tile(
                                [128, 1024], dt.float32, tag=f"t{tau}",
                                name=f"t{tau}p{(pa // 2) % 2}")
                        # x pass first: full-bank identity matmul (N=480)
                        for pa, pb in pairs:
                            for s in range(pa, pb):
                                slot = (s - pa) * 512
                                nc.tensor.matmul(
                                    ptiles[tau, pa][:, slot: slot + 480],
                                    lhsT=id_sb[:],
                                    rhs=xt[tau][:, s * SUB * WP8:
                                                s * SUB * WP8 + 480],
                                    start=True, stop=(t == 0))
                        if t > 0:
                            for dx in range(3):
                                for pa, pb in pairs:
                                    for s in range(pa, pb):
                                        slot = (s - pa) * 512
                                        rhs = _replace_ap(
                                            hbr[:, s * SUB: s * SUB + 2, :],
                                            [[HR8 * WP8, 128], [WP8, 2],
                                             [1, 480]],
                                            extra_offset=dx)
                                        nc.tensor.matmul(
                                            ptiles[tau, pa][:,
                                                            slot: slot + 480],
                                            lhsT=w_sb[:, tau * 3 + dx],
                                            rhs=rhs,
                                            start=False, stop=(dx == 2),
                                            perf_mode=PM.DoubleRow)
                    # gate activations (4D strided psum read, compact out)
                    for tau in range(2):
                        dst = sa if tau == 0 else sb
                        for pa, pb in pairs:
                            n = pb - pa
                            pt = ptiles[tau, pa]
                            if n == 2:
                                in_ap = _replace_ap(
                                    pt[:, 0:768],
                                    [[1024, 128], [512, 2], [WP8, SUB],
                                     [1, W]])
                            else:
                                in_ap = _replace_ap(
                                    pt[:, 0:384],
                                    [[1024, 128], [WP8, SUB], [1, W]])
                            o = pa * SUB * W
                            if tau == 0:
                                nc.scalar.activation(
                                    dst[:, o: o + n * SUB * W], in_ap,
                                    Act.Sigmoid, scale=1.0 / 16.0)
                            else:
                                nc.scalar.activation(
                                    dst[:, o: o + n * SUB * W], in_ap,
                                    Act.Tanh, scale=sc_sb[:])

                    # elementwise span
                    a0 = sp0 * SUB * W
                    a1 = sp1 * SUB * W
                    seg = slice(a0, a1)
                    r0 = sp0 * SUB
                    r1 = sp1 * SUB
                    # i*g (partition up-remap), f*c, fold to c_new
                    nc.vector.tensor_tensor(pp[64:128, seg], sa[0:64, seg],
                                            sb[0:64, seg], Alu.mult)
                    nc.vector.tensor_tensor(qq[64:128, seg], sa[64:128, seg],
                                            cs[64:128, seg], Alu.mult)
                    nc.vector.tensor_tensor(cs[64:128, seg], pp[64:128, seg],
                                            qq[64:128, seg], Alu.add)
                    nc.scalar.activation(tch[64:128, seg], cs[64:128, seg],
                                         Act.Tanh, scale=1.0)
                    # Hs = (so + 1) * tanh(c), into hw rows r0+1..r1+1
                    nc.vector.scalar_tensor_tensor(
                        hw[0:64, 1 + r0: 1 + r1, :],
                        sb[64:128, seg], 1.0, tch[64:128, seg],
                        Alu.add, Alu.mult)
                    if t < T_STEPS - 1:
                        rr1 = min(r1, R)
                        nc.vector.tensor_copy(
                            hw[64:128, r0: rr1, :],
                            hw[0:64, r0 + 1: rr1 + 1, :])
                        nc.gpsimd.tensor_copy(
                            hbw_t[:, r0: rr1, 1: 1 + W],
                            hw[:, r0: rr1, :])

                nc.sync.dma_start(out=hout[t],
                                  in_=hw[0:64, 1: 1 + OWN, :])

    nc.finalize()
    return nc


def _prep_inputs(x, w_h2h):
    """Per-core input maps. Cores: core = b*2 + half."""
    # channel order [i(0:64), f(64:128), g(192:256), o(128:192)]
    perm = np.concatenate([np.arange(0, 128), np.arange(192, 256),
                           np.arange(128, 192)])
    w_eff = (w_h2h.astype(np.float32) * 8.0)[perm]  # [256, 64, 3, 3]

    def pack_w(weff):
        wdr = np.zeros((128, 6, 2, 128), np.float32)
        for tau in range(2):
            blk = weff[tau * 128: (tau + 1) * 128]  # [128oc, 64ic, 3dy, 3dx]
            for dx in range(3):
                wdr[0:64, tau * 3 + dx, 0, :] = blk[:, :, 0, dx].T
                wdr[0:64, tau * 3 + dx, 1, :] = blk[:, :, 1, dx].T
                wdr[64:128, tau * 3 + dx, 1, :] = blk[:, :, 2, dx].T
        return wdr.reshape(128, 6 * 2 * 128).astype(float8_e4m3)

    w_top = pack_w(w_eff)
    w_bot = pack_w(w_eff[:, :, ::-1, :])
    ident = np.eye(128, dtype=np.float32).astype(bfloat16)
    scB = np.concatenate([np.full((64, 1), 1.0 / 16.0, np.float32),
                          np.full((64, 1), 1.0 / 32.0, np.float32)])

    xp = x[:, :, perm] * np.float32(16.0)  # [T, B, 256, H, W]
    in_maps = []
    for b in range(B):
        for half in range(2):
            if half == 0:
                xs = xp[:, b, :, 0:ROWS, :]
            else:
                xs = xp[:, b, :, H - ROWS:, :][:, :, ::-1, :]
            xs = np.ascontiguousarray(xs).astype(bfloat16)
            xpad = np.zeros((T_STEPS, 256, ROWS, WP8), bfloat16)
            xpad[:, :, :, 0:W] = xs
            xpad = xpad.reshape(T_STEPS, 2, 128, ROWS * WP8)
            in_maps.append({
                "x": xpad,
                "wdr": w_top if half == 0 else w_bot,
                "ident": ident,
                "scB": scB,
            })
    return in_maps


def kernel(x, w_h2h):
    from concourse import bass_utils

    if "nc" not in _CACHE:
        _CACHE["nc"] = _build_nc()
    nc = _CACHE["nc"]

    in_maps = _prep_inputs(np.asarray(x), np.asarray(w_h2h))
    res = bass_utils.run_bass_kernel_spmd(nc, in_maps,
                                          core_ids=list(range(8)))
    _CACHE["last_results"] = res

    out = np.zeros((T_STEPS, B, HIDDEN, H, W), np.float32)
    for b in range(B):
        for half in range(2):
            core = b * 2 + half
            hs = res.results[core]["hout"].astype(np.float32) * 0.5
            hs = hs.reshape(T_STEPS, HIDDEN, OWN, W)
            if half == 0:
                out[:, b, :, 0:OWN, :] = hs
            else:
                out[:, b, :, OWN:, :] = hs[:, :, ::-1, :]
    return out


# revision 20
# speedup vs baseline: 342.9378x; 342.9378x over previous
"""ConvLSTM cell kernel for Trainium2 (8 NeuronCores).

Sharding: data-parallel over batch B=4 x spatial split of H=64 into 2 halves
(8 shards). The recurrence prevents sharding T. Each core computes its half
with a shrinking row margin (47-t rows at step t) so no cross-core
communication is needed. Bottom halves are row-flipped on the host (x rows
flipped + conv kernel dy-flipped) so a single SPMD program serves all 8 cores.

Compute scheme (per core, per step):
  - h state lives in SBUF twice: bf16 `hw` [128, 50, 64] (parts 0:64 row r =
    Hs[r-1], parts 64:128 row r = Hs[r], Hs = 2*h) for output + conversion,
    and fp8e4 `hb` [128, 50, 80] (same row mapping, zero pad col 0, zero
    cols 65..79) used as conv rhs. Conv weights are pre-scaled by 8
    (= (w/2)*16) in fp8, x pre-scaled by 16 in bf16 and pre-padded to the
    80-col layout on the host, so psum = 16*z exactly.
  - 3x3 conv via fp8 DoubleRow matmuls: contraction 128 partitions x Ko=2
    (Ko stride = one 80-col padded row). Slots: (p<64,j0)=dy0, (p<64,j1)=dy1,
    (p>=64,j0)=dup(zero weight), (p>=64,j1)=dy2. One matmul per dx (3) with a
    flat 480-wide rhs span (6 output rows x 80 incl. junk pad cols), plus one
    bf16 identity matmul adding the padded x.
  - Gates: tau0 psum = [i;f] -> sigmoid (scale 1/16). tau1 psum = [g;o] ->
    tanh with per-partition scale [1/16; 1/32] giving [g; so=tanh(zo/2)].
    ACT reads psum via 4D strided APs (two 6-row subs per 2-bank psum tile),
    writing compact bf16 gate tiles. DVE: P = [i;f]*[g;c] (tensor_tensor, 2x
    mode), copy+add folds to c_new = i*g + f*c, ACT tanh(c_new), then one
    scalar_tensor_tensor Hs = (so+1)*tanh(c) = 2*o*tanh(c). Host multiplies
    the output by 0.5.
  - GpSimd converts [h;h_shifted] bf16 -> fp8 conv tiles; a SBUF->SBUF DMA
    builds the row-shifted bf16 half.
"""

import sys
import dataclasses

sys.path.insert(0, "/opt/trn_rl_repo")

import numpy as np
from ml_dtypes import bfloat16, float8_e4m3

HIDDEN = 64
T_STEPS = 16
B = 4
H = 64
W = 64
ROWS = 48         # per-core x rows (32 owned + 16 margin)
OWN = 32
WP8 = 80          # padded row width (DoubleRow Ko stride must be %16 bytes)
HR8 = 50          # h tile rows (top pad + 48 + bottom guard)
SUB = 6           # output rows per matmul (6*80=480 <= 512 fp32 psum bank)
CHUNK = 4         # subs per elementwise chunk (24 rows)

_CACHE = {}


def _replace_ap(ap, new_dims, extra_offset=0):
    a = ap.copy()
    return dataclasses.replace(a, ap=type(a.ap)(new_dims),
                               offset=a.offset + extra_offset)


def _build_nc():
    from concourse import bacc, mybir
    from concourse.tile import TileContext

    dt = mybir.dt
    Alu = mybir.AluOpType
    Act = mybir.ActivationFunctionType
    PM = mybir.MatmulPerfMode

    nc = bacc.Bacc(None, target_bir_lowering=False)

    XW = ROWS * W + 16   # compact x row span + overlap-window tail pad
    x_in = nc.dram_tensor("x", [T_STEPS, 128, 2 * XW], dt.bfloat16,
                          kind="ExternalInput")
    w_in = nc.dram_tensor("wdr", [128, 6 * 2 * 128], dt.float8e4,
                          kind="ExternalInput")
    id_in = nc.dram_tensor("ident", [128, 128], dt.bfloat16,
                           kind="ExternalInput")
    sc_in = nc.dram_tensor("scB", [128, 1], dt.float32,
                           kind="ExternalInput")
    ho_so = nc.dram_tensor("hout_so", [T_STEPS, 64, OWN * W], dt.bfloat16,
                           kind="ExternalOutput")
    ho_tc = nc.dram_tensor("hout_tc", [T_STEPS, 64, OWN * W], dt.bfloat16,
                           kind="ExternalOutput")

    NSUBMAX = ROWS // SUB            # 8
    GW = NSUBMAX * SUB * W           # 3072 gate-tile width

    with TileContext(nc) as tc:
        with (
            tc.tile_pool(name="const", bufs=1) as cpool,
            tc.tile_pool(name="state", bufs=1) as spool,
            tc.tile_pool(name="xload", bufs=2) as xpool,
            tc.tile_pool(name="work", bufs=2) as wpool,
            tc.tile_pool(name="ps", bufs=2, space="PSUM") as psp,
        ):
            w_sb = cpool.tile([128, 6, 2, 128], dt.float8e4, tag="w")
            id_sb = cpool.tile([128, 128], dt.bfloat16, tag="id")
            sc_sb = cpool.tile([128, 1], dt.float32, tag="sc")
            nc.sync.dma_start(out=w_sb[:], in_=w_in[:].rearrange(
                "p (a b c) -> p a b c", a=6, b=2))
            nc.sync.dma_start(out=id_sb[:], in_=id_in[:])
            nc.sync.dma_start(out=sc_sb[:], in_=sc_in[:])

            # fp8 conv-input state, ping-pong across steps
            hb = [spool.tile([128, HR8, WP8], dt.float8e4, tag=f"hb{k}",
                             name=f"hb{k}") for k in range(2)]
            nc.gpsimd.memset(hb[0][:], 0.0)
            nc.gpsimd.memset(hb[1][:], 0.0)
            # bf16 h tiles (ping-pong): parts 0:64 row r = Hs[r-1],
            # parts 64:128 row r = Hs[r]
            hwp = [spool.tile([128, HR8, W], dt.bfloat16, tag=f"hw{k}",
                              name=f"hw{k}") for k in range(2)]
            nc.gpsimd.memset(hwp[0][:], 0.0)
            nc.gpsimd.memset(hwp[1][:], 0.0)
            # c state (parts 64:128) + per-step g scratch (parts 0:64)
            cs = spool.tile([128, GW], dt.bfloat16, tag="cs")
            nc.gpsimd.memset(cs[:], 0.0)

            for t in range(T_STEPS):
                R = 47 - t
                nsub = -(-R // SUB)
                nrows = nsub * SUB
                hbr = hb[(t + 1) % 2]   # conv input (written at t-1)
                hbw_t = hb[t % 2]       # conv input for t+1 (written now)
                hw = hwp[t % 2]

                xti = xpool.tile([128, 2, XW], dt.bfloat16, tag="x",
                                 name="x")
                M = nrows * W + 16
                nc.sync.dma_start(
                    out=xti[:, :, 0:M],
                    in_=x_in[t].rearrange("p (a b) -> p a b", a=2)[:, :, 0:M])

                sa = wpool.tile([128, GW], dt.bfloat16, tag="sa", name="sa")
                sb = wpool.tile([128, GW], dt.bfloat16, tag="sb", name="sb")
                pp = wpool.tile([128, GW], dt.bfloat16, tag="pp", name="pp")
                qq = wpool.tile([128, GW], dt.bfloat16, tag="qq", name="qq")
                tch = wpool.tile([128, GW], dt.bfloat16, tag="tc", name="tc")

                # spans: first two small (12 rows) to release the next
                # step's rows early, remainder bigger for op efficiency
                if nsub > 4:
                    spans = [(0, 2), (2, 4), (4, nsub)]
                else:
                    spans = [(0, 2), (2, nsub)]
                for sp0, sp1 in spans:
                    pairs = [(pa, min(pa + 2, sp1))
                             for pa in range(sp0, sp1, 2)]
                    ptiles = {}
                    for tau in range(2):
                        for pa, pb in pairs:
                            ptiles[tau, pa] = psp.tile(
                                [128, 1024], dt.float32, tag=f"t{tau}",
                                name=f"t{tau}p{(pa // 2) % 2}")
                        # x pass first: full-bank identity matmul (N=480)
                        # via overlapping-window AP on the compact x tile
                        for pa, pb in pairs:
                            for s in range(pa, pb):
                                slot = (s - pa) * 512
                                rhs = _replace_ap(
                                    xti[:, tau, 0:480],
                                    [[2 * XW, 128], [W, SUB], [1, WP8]],
                                    extra_offset=s * SUB * W)
                                nc.tensor.matmul(
                                    ptiles[tau, pa][:, slot: slot + 480],
                                    lhsT=id_sb[:], rhs=rhs,
                                    start=True, stop=(t == 0))
                        if t > 0:
                            for dx in range(3):
                                for pa, pb in pairs:
                                    for s in range(pa, pb):
                                        slot = (s - pa) * 512
                                        rhs = _replace_ap(
                                            hbr[:, s * SUB: s * SUB + 2, :],
                                            [[HR8 * WP8, 128], [WP8, 2],
                                             [1, 480]],
                                            extra_offset=dx)
                                        nc.tensor.matmul(
                                            ptiles[tau, pa][:,
                                                            slot: slot + 480],
                                            lhsT=w_sb[:, tau * 3 + dx],
                                            rhs=rhs,
                                            start=False, stop=(dx == 2),
                                            perf_mode=PM.DoubleRow)
                    # gate activations (4D strided psum read, compact out)
                    for tau in range(2):
                        dst = sa if tau == 0 else sb
                        for pa, pb in pairs:
                            n = pb - pa
                            pt = ptiles[tau, pa]
                            if n == 2:
                                in_ap = _replace_ap(
                                    pt[:, 0:768],
                                    [[1024, 128], [512, 2], [WP8, SUB],
                                     [1, W]])
                            else:
                                in_ap = _replace_ap(
                                    pt[:, 0:384],
                                    [[1024, 128], [WP8, SUB], [1, W]])
                            o = pa * SUB * W
                            if tau == 0:
                                nc.scalar.activation(
                                    dst[:, o: o + n * SUB * W], in_ap,
                                    Act.Sigmoid, scale=1.0 / 16.0)
                            else:
                                nc.scalar.activation(
                                    dst[:, o: o + n * SUB * W], in_ap,
                                    Act.Tanh, scale=sc_sb[:])

                    # elementwise span
                    a0 = sp0 * SUB * W
                    a1 = sp1 * SUB * W
                    seg = slice(a0, a1)
                    r0 = sp0 * SUB
                    r1 = sp1 * SUB
                    # i*g (partition up-remap), f*c, fold to c_new
                    nc.vector.tensor_tensor(pp[64:128, seg], sa[0:64, seg],
                                            sb[0:64, seg], Alu.mult)
                    nc.vector.tensor_tensor(qq[64:128, seg], sa[64:128, seg],
                                            cs[64:128, seg], Alu.mult)
                    nc.vector.tensor_tensor(cs[64:128, seg], pp[64:128, seg],
                                            qq[64:128, seg], Alu.add)
                    nc.scalar.activation(tch[64:128, seg], cs[64:128, seg],
                                         Act.Tanh, scale=1.0)
                    # Hs = (so + 1) * tanh(c) -> bf16 h tile, then the
                    # shifted half (DVE 4x copy) and fp8 convert (GpSimd)
                    if t < T_STEPS - 1:
                        nc.vector.scalar_tensor_tensor(
                            hw[0:64, 1 + r0: 1 + r1, :],
                            sb[64:128, seg], 1.0, tch[64:128, seg],
                            Alu.add, Alu.mult)
                        rr1 = min(r1, R)
                        nc.vector.tensor_copy(
                            hw[64:128, r0: rr1, :],
                            hw[0:64, r0 + 1: rr1 + 1, :])
                        nc.gpsimd.tensor_copy(
                            hbw_t[:, r0: rr1, 1: 1 + W],
                            hw[:, r0: rr1, :])

                # host combines h = 0.5*(so+1)*tanh(c)
                nc.sync.dma_start(out=ho_so[t],
                                  in_=sb[64:128, 0: OWN * W])
                nc.sync.dma_start(out=ho_tc[t],
                                  in_=tch[64:128, 0: OWN * W])

    nc.finalize()
    return nc


def _prep_inputs(x, w_h2h):
    """Per-core input maps. Cores: core = b*2 + half."""
    # channel order [i(0:64), f(64:128), g(192:256), o(128:192)]
    perm = np.concatenate([np.arange(0, 128), np.arange(192, 256),
                           np.arange(128, 192)])
    w_eff = (w_h2h.astype(np.float32) * 8.0)[perm]  # [256, 64, 3, 3]

    def pack_w(weff):
        wdr = np.zeros((128, 6, 2, 128), np.float32)
        for tau in range(2):
            blk = weff[tau * 128: (tau + 1) * 128]  # [128oc, 64ic, 3dy, 3dx]
            for dx in range(3):
                wdr[0:64, tau * 3 + dx, 0, :] = blk[:, :, 0, dx].T
                wdr[0:64, tau * 3 + dx, 1, :] = blk[:, :, 1, dx].T
                wdr[64:128, tau * 3 + dx, 1, :] = blk[:, :, 2, dx].T
        return wdr.reshape(128, 6 * 2 * 128).astype(float8_e4m3)

    w_top = pack_w(w_eff)
    w_bot = pack_w(w_eff[:, :, ::-1, :])
    ident = np.eye(128, dtype=np.float32).astype(bfloat16)
    scB = np.concatenate([np.full((64, 1), 1.0 / 16.0, np.float32),
                          np.full((64, 1), 1.0 / 32.0, np.float32)])

    xp = x[:, :, perm] * np.float32(16.0)  # [T, B, 256, H, W]
    in_maps = []
    for b in range(B):
        for half in range(2):
            if half == 0:
                xs = xp[:, b, :, 0:ROWS, :]
            else:
                xs = xp[:, b, :, H - ROWS:, :][:, :, ::-1, :]
            xs = np.ascontiguousarray(xs).astype(bfloat16)
            # [T, 128, 2, ROWS*W+16]: partition-major, per-tau compact rows
            # with a 16-elem tail pad for the overlapping matmul window
            XW = ROWS * W + 16
            xpad = np.zeros((T_STEPS, 128, 2, XW), bfloat16)
            xf = xs.reshape(T_STEPS, 2, 128, ROWS * W)
            xpad[:, :, 0, : ROWS * W] = xf[:, 0]
            xpad[:, :, 1, : ROWS * W] = xf[:, 1]
            xpad = xpad.reshape(T_STEPS, 128, 2 * XW)
            in_maps.append({
                "x": xpad,
                "wdr": w_top if half == 0 else w_bot,
                "ident": ident,
                "scB": scB,
            })
    return in_maps


def kernel(x, w_h2h):
    from concourse import bass_utils

    if "nc" not in _CACHE:
        _CACHE["nc"] = _build_nc()
    nc = _CACHE["nc"]

    in_maps = _prep_inputs(np.asarray(x), np.asarray(w_h2h))
    res = bass_utils.run_bass_kernel_spmd(nc, in_maps,
                                          core_ids=list(range(8)))
    _CACHE["last_results"] = res

    out = np.zeros((T_STEPS, B, HIDDEN, H, W), np.float32)
    for b in range(B):
        for half in range(2):
            core = b * 2 + half
            so = res.results[core]["hout_so"].astype(np.float32)
            tc = res.results[core]["hout_tc"].astype(np.float32)
            hs = (0.5 * (so + 1.0) * tc).reshape(T_STEPS, HIDDEN, OWN, W)
            if half == 0:
                out[:, b, :, 0:OWN, :] = hs
            else:
                out[:, b, :, OWN:, :] = hs[:, :, ::-1, :]
    return out


# revision 21
# speedup vs baseline: 348.3171x; 1.0157x over previous
"""ConvLSTM cell kernel for Trainium2 (8 NeuronCores).

Sharding: data-parallel over batch B=4 x spatial split of H=64 into 2 halves
(8 shards). The recurrence prevents sharding T. Each core computes its half
with a shrinking row margin (47-t rows at step t) so no cross-core
communication is needed. Bottom halves are row-flipped on the host (x rows
flipped + conv kernel dy-flipped) so a single SPMD program serves all 8 cores.

Compute scheme (per core, per step):
  - h state lives in SBUF twice: bf16 `hw` [128, 50, 64] (parts 0:64 row r =
    Hs[r-1], parts 64:128 row r = Hs[r], Hs = 2*h) for output + conversion,
    and fp8e4 `hb` [128, 50, 80] (same row mapping, zero pad col 0, zero
    cols 65..79) used as conv rhs. Conv weights are pre-scaled by 8
    (= (w/2)*16) in fp8, x pre-scaled by 16 in bf16 and pre-padded to the
    80-col layout on the host, so psum = 16*z exactly.
  - 3x3 conv via fp8 DoubleRow matmuls: contraction 128 partitions x Ko=2
    (Ko stride = one 80-col padded row). Slots: (p<64,j0)=dy0, (p<64,j1)=dy1,
    (p>=64,j0)=dup(zero weight), (p>=64,j1)=dy2. One matmul per dx (3) with a
    flat 480-wide rhs span (6 output rows x 80 incl. junk pad cols), plus one
    bf16 identity matmul adding the padded x.
  - Gates: tau0 psum = [i;f] -> sigmoid (scale 1/16). tau1 psum = [g;o] ->
    tanh with per-partition scale [1/16; 1/32] giving [g; so=tanh(zo/2)].
    ACT reads psum via 4D strided APs (two 6-row subs per 2-bank psum tile),
    writing compact bf16 gate tiles. DVE: P = [i;f]*[g;c] (tensor_tensor, 2x
    mode), copy+add folds to c_new = i*g + f*c, ACT tanh(c_new), then one
    scalar_tensor_tensor Hs = (so+1)*tanh(c) = 2*o*tanh(c). Host multiplies
    the output by 0.5.
  - GpSimd converts [h;h_shifted] bf16 -> fp8 conv tiles; a SBUF->SBUF DMA
    builds the row-shifted bf16 half.
"""

import sys
import dataclasses

sys.path.insert(0, "/opt/trn_rl_repo")

import numpy as np
from ml_dtypes import bfloat16, float8_e4m3

HIDDEN = 64
T_STEPS = 16
B = 4
H = 64
W = 64
ROWS = 48         # per-core x rows (32 owned + 16 margin)
OWN = 32
WP8 = 80          # padded row width (DoubleRow Ko stride must be %16 bytes)
HR8 = 50          # h tile rows (top pad + 48 + bottom guard)
SUB = 6           # output rows per matmul (6*80=480 <= 512 fp32 psum bank)
CHUNK = 4         # subs per elementwise chunk (24 rows)

_CACHE = {}


def _replace_ap(ap, new_dims, extra_offset=0):
    a = ap.copy()
    return dataclasses.replace(a, ap=type(a.ap)(new_dims),
                               offset=a.offset + extra_offset)


def _build_nc():
    from concourse import bacc, mybir
    from concourse.tile import TileContext

    dt = mybir.dt
    Alu = mybir.AluOpType
    Act = mybir.ActivationFunctionType
    PM = mybir.MatmulPerfMode

    nc = bacc.Bacc(None, target_bir_lowering=False)

    XW = ROWS * W + 16   # compact x row span + overlap-window tail pad
    x_in = nc.dram_tensor("x", [T_STEPS, 128, 2 * XW], dt.bfloat16,
                          kind="ExternalInput")
    w_in = nc.dram_tensor("wdr", [128, 6 * 2 * 128], dt.float8e4,
                          kind="ExternalInput")
    id_in = nc.dram_tensor("ident", [128, 128], dt.bfloat16,
                           kind="ExternalInput")
    sc_in = nc.dram_tensor("scB", [128, 1], dt.float32,
                           kind="ExternalInput")
    ho_so = nc.dram_tensor("hout_so", [T_STEPS, 64, OWN * W], dt.bfloat16,
                           kind="ExternalOutput")
    ho_tc = nc.dram_tensor("hout_tc", [T_STEPS, 64, OWN * W], dt.bfloat16,
                           kind="ExternalOutput")

    NSUBMAX = ROWS // SUB            # 8
    GW = NSUBMAX * SUB * W           # 3072 gate-tile width

    with TileContext(nc) as tc:
        with (
            tc.tile_pool(name="const", bufs=1) as cpool,
            tc.tile_pool(name="state", bufs=1) as spool,
            tc.tile_pool(name="xload", bufs=2) as xpool,
            tc.tile_pool(name="work", bufs=2) as wpool,
            tc.tile_pool(name="ps", bufs=2, space="PSUM") as psp,
        ):
            w_sb = cpool.tile([128, 6, 2, 128], dt.float8e4, tag="w")
            id_sb = cpool.tile([128, 128], dt.bfloat16, tag="id")
            sc_sb = cpool.tile([128, 1], dt.float32, tag="sc")
            nc.sync.dma_start(out=w_sb[:], in_=w_in[:].rearrange(
                "p (a b c) -> p a b c", a=6, b=2))
            nc.sync.dma_start(out=id_sb[:], in_=id_in[:])
            nc.sync.dma_start(out=sc_sb[:], in_=sc_in[:])

            # fp8 conv-input state, ping-pong across steps
            hb = [spool.tile([128, HR8, WP8], dt.float8e4, tag=f"hb{k}",
                             name=f"hb{k}") for k in range(2)]
            nc.gpsimd.memset(hb[0][:], 0.0)
            nc.gpsimd.memset(hb[1][:], 0.0)
            # c state (parts 64:128) + per-step g scratch (parts 0:64)
            cs = spool.tile([128, GW], dt.bfloat16, tag="cs")
            nc.gpsimd.memset(cs[:], 0.0)

            for t in range(T_STEPS):
                R = 47 - t
                nsub = -(-R // SUB)
                nrows = nsub * SUB
                hbr = hb[(t + 1) % 2]   # conv input (written at t-1)
                hbw_t = hb[t % 2]       # conv input for t+1 (written now)

                xti = xpool.tile([128, 2, XW], dt.bfloat16, tag="x",
                                 name="x")
                M = nrows * W + 16
                nc.sync.dma_start(
                    out=xti[:, :, 0:M],
                    in_=x_in[t].rearrange("p (a b) -> p a b", a=2)[:, :, 0:M])

                sa = wpool.tile([128, GW], dt.bfloat16, tag="sa", name="sa")
                sb = wpool.tile([128, GW], dt.bfloat16, tag="sb", name="sb")
                pp = wpool.tile([128, GW], dt.bfloat16, tag="pp", name="pp")
                qq = wpool.tile([128, GW], dt.bfloat16, tag="qq", name="qq")
                tch = wpool.tile([128, GW], dt.bfloat16, tag="tc", name="tc")

                # spans: first two small (12 rows) to release the next
                # step's rows early, remainder bigger for op efficiency
                if nsub > 4:
                    spans = [(0, 2), (2, 4), (4, nsub)]
                else:
                    spans = [(0, 2), (2, nsub)]
                for sp0, sp1 in spans:
                    pairs = [(pa, min(pa + 2, sp1))
                             for pa in range(sp0, sp1, 2)]
                    ptiles = {}
                    for tau in range(2):
                        for pa, pb in pairs:
                            ptiles[tau, pa] = psp.tile(
                                [128, 1024], dt.float32, tag=f"t{tau}",
                                name=f"t{tau}p{(pa // 2) % 2}")
                        # x pass first: full-bank identity matmul (N=480)
                        # via overlapping-window AP on the compact x tile
                        for pa, pb in pairs:
                            for s in range(pa, pb):
                                slot = (s - pa) * 512
                                rhs = _replace_ap(
                                    xti[:, tau, 0:480],
                                    [[2 * XW, 128], [W, SUB], [1, WP8]],
                                    extra_offset=s * SUB * W)
                                nc.tensor.matmul(
                                    ptiles[tau, pa][:, slot: slot + 480],
                                    lhsT=id_sb[:], rhs=rhs,
                                    start=True, stop=(t == 0))
                        if t > 0:
                            for dx in range(3):
                                for pa, pb in pairs:
                                    for s in range(pa, pb):
                                        slot = (s - pa) * 512
                                        rhs = _replace_ap(
                                            hbr[:, s * SUB: s * SUB + 2, :],
                                            [[HR8 * WP8, 128], [WP8, 2],
                                             [1, 480]],
                                            extra_offset=dx)
                                        nc.tensor.matmul(
                                            ptiles[tau, pa][:,
                                                            slot: slot + 480],
                                            lhsT=w_sb[:, tau * 3 + dx],
                                            rhs=rhs,
                                            start=False, stop=(dx == 2),
                                            perf_mode=PM.DoubleRow)
                    # gate activations (4D strided psum read, compact out)
                    for tau in range(2):
                        dst = sa if tau == 0 else sb
                        for pa, pb in pairs:
                            n = pb - pa
                            pt = ptiles[tau, pa]
                            if n == 2:
                                in_ap = _replace_ap(
                                    pt[:, 0:768],
                                    [[1024, 128], [512, 2], [WP8, SUB],
                                     [1, W]])
                            else:
                                in_ap = _replace_ap(
                                    pt[:, 0:384],
                                    [[1024, 128], [WP8, SUB], [1, W]])
                            o = pa * SUB * W
                            if tau == 0:
                                nc.scalar.activation(
                                    dst[:, o: o + n * SUB * W], in_ap,
                                    Act.Sigmoid, scale=1.0 / 16.0)
                            else:
                                nc.scalar.activation(
                                    dst[:, o: o + n * SUB * W], in_ap,
                                    Act.Tanh, scale=sc_sb[:])

                    # elementwise span
                    a0 = sp0 * SUB * W
                    a1 = sp1 * SUB * W
                    seg = slice(a0, a1)
                    r0 = sp0 * SUB
                    r1 = sp1 * SUB
                    # i*g (partition up-remap), f*c, fold to c_new
                    nc.vector.tensor_tensor(pp[64:128, seg], sa[0:64, seg],
                                            sb[0:64, seg], Alu.mult)
                    nc.vector.tensor_tensor(qq[64:128, seg], sa[64:128, seg],
                                            cs[64:128, seg], Alu.mult)
                    nc.vector.tensor_tensor(cs[64:128, seg], pp[64:128, seg],
                                            qq[64:128, seg], Alu.add)
                    nc.scalar.activation(tch[64:128, seg], cs[64:128, seg],
                                         Act.Tanh, scale=1.0)
                    # Hs = (so + 1) * tanh(c), written fp8 directly into
                    # both halves of the next conv tile (low: row r+1 holds
                    # Hs[r]; high: row r holds Hs[r])
                    if t < T_STEPS - 1:
                        nc.vector.scalar_tensor_tensor(
                            hbw_t[0:64, 1 + r0: 1 + r1, 1: 1 + W],
                            sb[64:128, seg], 1.0, tch[64:128, seg],
                            Alu.add, Alu.mult)
                        nc.vector.scalar_tensor_tensor(
                            hbw_t[64:128, r0: r1, 1: 1 + W],
                            sb[64:128, seg], 1.0, tch[64:128, seg],
                            Alu.add, Alu.mult)

                # host combines h = 0.5*(so+1)*tanh(c)
                nc.sync.dma_start(out=ho_so[t],
                                  in_=sb[64:128, 0: OWN * W])
                nc.sync.dma_start(out=ho_tc[t],
                                  in_=tch[64:128, 0: OWN * W])

    nc.finalize()
    return nc


def _prep_inputs(x, w_h2h):
    """Per-core input maps. Cores: core = b*2 + half."""
    # channel order [i(0:64), f(64:128), g(192:256), o(128:192)]
    perm = np.concatenate([np.arange(0, 128), np.arange(192, 256),
                           np.arange(128, 192)])
    w_eff = (w_h2h.astype(np.float32) * 8.0)[perm]  # [256, 64, 3, 3]

    def pack_w(weff):
        wdr = np.zeros((128, 6, 2, 128), np.float32)
        for tau in range(2):
            blk = weff[tau * 128: (tau + 1) * 128]  # [128oc, 64ic, 3dy, 3dx]
            for dx in range(3):
                wdr[0:64, tau * 3 + dx, 0, :] = blk[:, :, 0, dx].T
                wdr[0:64, tau * 3 + dx, 1, :] = blk[:, :, 1, dx].T
                wdr[64:128, tau * 3 + dx, 1, :] = blk[:, :, 2, dx].T
        return wdr.reshape(128, 6 * 2 * 128).astype(float8_e4m3)

    w_top = pack_w(w_eff)
    w_bot = pack_w(w_eff[:, :, ::-1, :])
    ident = np.eye(128, dtype=np.float32).astype(bfloat16)
    scB = np.concatenate([np.full((64, 1), 1.0 / 16.0, np.float32),
                          np.full((64, 1), 1.0 / 32.0, np.float32)])

    xp = x[:, :, perm] * np.float32(16.0)  # [T, B, 256, H, W]
    in_maps = []
    for b in range(B):
        for half in range(2):
            if half == 0:
                xs = xp[:, b, :, 0:ROWS, :]
            else:
                xs = xp[:, b, :, H - ROWS:, :][:, :, ::-1, :]
            xs = np.ascontiguousarray(xs).astype(bfloat16)
            # [T, 128, 2, ROWS*W+16]: partition-major, per-tau compact rows
            # with a 16-elem tail pad for the overlapping matmul window
            XW = ROWS * W + 16
            xpad = np.zeros((T_STEPS, 128, 2, XW), bfloat16)
            xf = xs.reshape(T_STEPS, 2, 128, ROWS * W)
            xpad[:, :, 0, : ROWS * W] = xf[:, 0]
            xpad[:, :, 1, : ROWS * W] = xf[:, 1]
            xpad = xpad.reshape(T_STEPS, 128, 2 * XW)
            in_maps.append({
                "x": xpad,
                "wdr": w_top if half == 0 else w_bot,
                "ident": ident,
                "scB": scB,
            })
    return in_maps


def kernel(x, w_h2h):
    from concourse import bass_utils

    if "nc" not in _CACHE:
        _CACHE["nc"] = _build_nc()
    nc = _CACHE["nc"]

    in_maps = _prep_inputs(np.asarray(x), np.asarray(w_h2h))
    res = bass_utils.run_bass_kernel_spmd(nc, in_maps,
                                          core_ids=list(range(8)))
    _CACHE["last_results"] = res

    out = np.zeros((T_STEPS, B, HIDDEN, H, W), np.float32)
    for b in range(B):
        for half in range(2):
            core = b * 2 + half
            so = res.results[core]["hout_so"].astype(np.float32)
            tc = res.results[core]["hout_tc"].astype(np.float32)
            hs = (0.5 * (so + 1.0) * tc).reshape(T_STEPS, HIDDEN, OWN, W)
            if half == 0:
                out[:, b, :, 0:OWN, :] = hs
            else:
                out[:, b, :, OWN:, :] = hs[:, :, ::-1, :]
    return out
